# revision 1
# baseline (speedup 1.0000x reference)
"""Causal self-attention (GQA, RoPE, QK-RMSNorm) Trainium2 Bass kernel.

Sharding: 8 cores = 2 batches x 4 KV groups. Core i handles batch i//4 and
KV group i%4 (4 query heads + 1 KV head). c_q/c_k/c_v column-sharded,
c_proj row-sharded; the partial output sums are reduced on the host.

Device-side layout trick: the host ships x^T (plus stacked cos/sin tables),
so every matmul contraction dim lands on SBUF partitions with zero on-device
transposes of x. Attention uses the "scores-transposed" formulation:
  s^T[tk, tq] = k^T.T @ q^T  (k block stationary)
  p^T = exp(s^T * scale)     (no max subtraction: QK-RMSNorm bounds |s*scale| <= sqrt(128))
  y[tq, d], r[tq] = p^T.T @ [v | 1]  (ones column gives the softmax denominator)
so softmax needs no partition-dim reductions and no probability transposes.
"""

import sys

sys.path.insert(0, "/opt/trn_rl_repo")

import numpy as np
import ml_dtypes

_BF16NP = ml_dtypes.bfloat16

import concourse.bacc as bacc
import concourse.tile as tile
import concourse.mybir as mybir
from concourse.bass_utils import run_bass_kernel_spmd

# Problem constants (hardcoded per contract)
B = 2
T = 2048
D = 2048
N_HEAD = 16
N_KV = 4
DH = 128
REP = N_HEAD // N_KV  # 4 query heads per KV head
HG = REP * DH  # 512 query columns per core
EPS = 1.1920928955078125e-07
SCALE = 1.0 / float(np.sqrt(DH))
MASK_VAL = -1e9

P = 128
TCH = 512  # T chunk (psum free dim)
NTCH = T // TCH  # 4
NDCH = D // P  # 16
NTB = T // P  # 16 Tk blocks

F32 = mybir.dt.float32
F32R = mybir.dt.float32r
BF16 = mybir.dt.bfloat16

N_CORES = 8

_CACHE = {}


def _build():
    nc = bacc.Bacc("TRN2", num_devices=N_CORES)
    xT = nc.dram_tensor("xT", [D, T], BF16, kind="ExternalInput").ap()
    cos2 = nc.dram_tensor("cos2", [P, T], BF16, kind="ExternalInput").ap()
    sin2n = nc.dram_tensor("sin2n", [P, T], BF16, kind="ExternalInput").ap()
    wq = nc.dram_tensor("wq", [D, HG], BF16, kind="ExternalInput").ap()
    wk = nc.dram_tensor("wk", [D, DH], BF16, kind="ExternalInput").ap()
    wv = nc.dram_tensor("wv", [D, DH], BF16, kind="ExternalInput").ap()
    wo = nc.dram_tensor("wo", [HG, D], BF16, kind="ExternalInput").ap()
    out = nc.dram_tensor("out", [T, D], F32, kind="ExternalOutput").ap()

    with tile.TileContext(nc) as tc:
        _trace(tc, xT, cos2, sin2n, wq, wk, wv, wo, out)
    nc.compile()
    return nc


def _trace(tc, xT, cos2, sin2n, wq, wk, wv, wo, out):
    nc = tc.nc
    from contextlib import ExitStack

    ctx = ExitStack()
    with ctx:
        # ---------------- pools ----------------
        const_pool = ctx.enter_context(tc.tile_pool(name="consts", bufs=1))
        attn_pool = ctx.enter_context(tc.tile_pool(name="attn", bufs=1))
        qr_pool = ctx.enter_context(tc.tile_pool(name="qr", bufs=2))
        xt_pool = ctx.enter_context(tc.tile_pool(name="xt", bufs=1))
        wkv_pool = ctx.enter_context(tc.tile_pool(name="wkv", bufs=1))
        wqs_pool = ctx.enter_context(tc.tile_pool(name="wqs", bufs=1))
        tab_pool = ctx.enter_context(tc.tile_pool(name="tab", bufs=1))
        pre_pool = ctx.enter_context(tc.tile_pool(name="pre", bufs=2))
        pt_pool = ctx.enter_context(tc.tile_pool(name="pt", bufs=1))
        yt_pool = ctx.enter_context(tc.tile_pool(name="yt", bufs=1))
        osb_pool = ctx.enter_context(tc.tile_pool(name="osb", bufs=2))
        # PSUM bank ledger (8 banks): proj 2 + s 2 + o 2 + y 1 + aux 1
        ps_proj = ctx.enter_context(tc.tile_pool(name="ps_proj", bufs=1, space="PSUM"))
        ps_aux = ctx.enter_context(tc.tile_pool(name="ps_aux", bufs=1, space="PSUM"))
        ps_s = ctx.enter_context(tc.tile_pool(name="ps_s", bufs=2, space="PSUM"))
        ps_o = ctx.enter_context(tc.tile_pool(name="ps_o", bufs=2, space="PSUM"))
        ps_y = ctx.enter_context(tc.tile_pool(name="ps_y", bufs=1, space="PSUM"))

        # ---------------- constants ----------------
        ident_f = const_pool.tile([P, P], F32)
        nc.gpsimd.memset(ident_f, 0.0)
        nc.gpsimd.affine_select(
            out=ident_f, in_=ident_f, compare_op=mybir.AluOpType.not_equal,
            fill=1.0, base=0, pattern=[[-1, P]], channel_multiplier=1,
        )
        ident = const_pool.tile([P, P], BF16)
        nc.vector.tensor_copy(out=ident, in_=ident_f)
        # reuse ident_f as the all-ones source once ident is copied out
        nc.vector.memset(ident_f, 1.0)
        onesm = const_pool.tile([P, P], BF16)
        nc.vector.tensor_copy(out=onesm, in_=ident_f)

        # additive causal mask for a diagonal 128x128 block: keep iff col >= row
        mask_sb = const_pool.tile([P, P], F32)
        nc.gpsimd.memset(mask_sb, 0.0)
        nc.gpsimd.affine_select(
            out=mask_sb, in_=mask_sb, compare_op=mybir.AluOpType.is_ge,
            fill=MASK_VAL, base=0, pattern=[[1, P]], channel_multiplier=-1,
        )
        eps_sb = const_pool.tile([P, 1], F32)
        nc.vector.memset(eps_sb, EPS)

        # ---------------- persistent operands ----------------
        krot = attn_pool.tile([P, T], BF16, tag="krot", name="krot")
        vaug = [attn_pool.tile([P, DH + 1], BF16, tag=f"vaug{m}", name=f"vaug{m}")
                for m in range(NTB)]
        wo_sb = attn_pool.tile([P, REP, D], BF16, name="wo_sb")
        wo_r = wo.rearrange("(n p) d -> p n d", p=P)

        wk_sb = wkv_pool.tile([P, NDCH, DH], BF16, name="wk_sb")
        wv_sb = wkv_pool.tile([P, NDCH, DH], BF16, name="wv_sb")
        wk_r = wk.rearrange("(n p) h -> p n h", p=P)
        wv_r = wv.rearrange("(n p) h -> p n h", p=P)
        wq_r = wq.rearrange("(n p) h -> p n h", p=P)
        xT_r = xT.rearrange("(i p) t -> p i t", p=P)
        nc.sync.dma_start(out=wk_sb[:, 0:2, :], in_=wk_r[:, 0:2, :])
        nc.sync.dma_start(out=wk_sb[:, 2:8, :], in_=wk_r[:, 2:8, :])

        def load_x_chunk(j, order=(0, 1)):
            halves = [None, None]
            for q in order:
                t = xt_pool.tile([P, 8, TCH], BF16, tag=f"xt{q}",
                                 name=f"xt{q}_{j}")
                nc.sync.dma_start(
                    out=t, in_=xT_r[:, 8 * q:8 * q + 8,
                                    j * TCH:(j + 1) * TCH])
                halves[q] = t
            return halves

        # per-chunk rope tables (double-buffered; full-T residency would
        # overflow SBUF with wq resident)

        # wq stays resident across all chunks: one 4MB load, staged in
        # halves per head during the j=0 passes.
        wq_sb = wqs_pool.tile([P, N_HEAD // N_KV * NDCH, DH], BF16, name="wq_sb")

        def load_wq_head(h, part):
            hsl = slice(h * DH, (h + 1) * DH)
            if part in (0, 2):
                nc.sync.dma_start(
                    out=wq_sb[:, h * NDCH:h * NDCH + 8, :],
                    in_=wq_r[:, 0:8, hsl])
            if part in (1, 2):
                nc.sync.dma_start(
                    out=wq_sb[:, h * NDCH + 8:h * NDCH + 16, :],
                    in_=wq_r[:, 8:16, hsl])

        # ---------------- phase A(j): projections + RMS/RoPE + v prep ----------
        # out-major passes over a resident x^T chunk set: 2 psum banks total.
        # pass order (k, v), (q0, q1), (q2, q3): krot first unblocks scores.
        # RMS rsqrt is batched once per chunk (Abs_reciprocal_sqrt lives in a
        # different act table than Exp/Square/Copy; per-pass use would reload
        # the 1.28us table twice per pass).
        def emit_A_kv(j, qr_tiles, dq, xch, xnext):
            stats_q = []
            cs = slice(j * TCH, (j + 1) * TCH)
            ms_all = pre_pool.tile([P, 5, TCH], BF16, tag="ms", bufs=1,
                                   name=f"ms_{j}")
            cos_t = tab_pool.tile([P, TCH], BF16, tag="cos", bufs=2,
                                  name=f"cos_{j}")
            sin_t = tab_pool.tile([P, TCH], BF16, tag="sin", bufs=2,
                                  name=f"sin_{j}")
            if j == 0:
                nc.sync.dma_start(out=wk_sb[:, 8:16, :], in_=wk_r[:, 8:16, :])
                nc.sync.dma_start(out=wv_sb[:, 0:8, :], in_=wv_r[:, 0:8, :])
                nc.sync.dma_start(out=wv_sb[:, 8:16, :], in_=wv_r[:, 8:16, :])
                nc.sync.dma_start(out=cos_t, in_=cos2[:, cs])
                nc.sync.dma_start(out=sin_t, in_=sin2n[:, cs])
                for h in range(REP):
                    load_wq_head(h, 2)
                nc.sync.dma_start(out=wo_sb[:, 0, :], in_=wo_r[:, 0, :])
                nc.sync.dma_start(out=wo_sb[:, 1, :], in_=wo_r[:, 1, :])
            else:
                nc.sync.dma_start(out=cos_t, in_=cos2[:, cs])
                nc.sync.dma_start(out=sin_t, in_=sin2n[:, cs])
                if j == 1:
                    nc.sync.dma_start(out=wo_sb[:, 2, :], in_=wo_r[:, 2, :])
                    nc.sync.dma_start(out=wo_sb[:, 3, :], in_=wo_r[:, 3, :])
            xts = [xch[i // 8][:, i % 8, :] for i in range(NDCH)]

            def rope_raw(idx, psum, dst):
                # stats: square + all-ones matmul -> per-column sums on every
                # row; park them in ms_all for the batched rsqrt.
                # evacuate the projection once; rope math then runs all-bf16
                # all-SBUF, which DVE executes at 2x
                qsb = pre_pool.tile([P, TCH], BF16, tag="qsb",
                                    name=f"qsb{idx}_{j}")
                nc.vector.tensor_copy(out=qsb, in_=psum)
                sq = pre_pool.tile([P, TCH], BF16, tag="sq", name=f"sq{idx}_{j}")
                nc.scalar.activation(
                    out=sq, in_=qsb, func=mybir.ActivationFunctionType.Square)

                def do_stats():
                    rps = ps_aux.tile([P, TCH], F32, tag="aux",
                                      name=f"rstd{idx}_{j}")
                    nc.tensor.matmul(rps, onesm, sq, start=True, stop=True)
                    slot = 0 if idx == 4 else 1 + idx
                    nc.vector.tensor_copy(out=ms_all[:, slot, :], in_=rps)

                # deferred into the next pass so the in-order PE queue never
                # parks on the Act square finishing
                stats_q.append(do_stats)
                nc.vector.tensor_mul(dst, qsb, cos_t)
                # rotate-half: inputs must share a base partition; sin2n ships
                # as [-sin; +sin] so only the output base differs.
                m2 = pre_pool.tile([P, TCH], BF16, tag="sq", name=f"m2_{idx}_{j}")
                nc.vector.tensor_mul(m2[0:64, :], qsb[64:128, :],
                                     sin_t[64:128, :])
                nc.vector.tensor_mul(m2[64:128, :], qsb[0:64, :],
                                     sin_t[0:64, :])
                nc.vector.tensor_add(dst, dst, m2)

            def rope_fin(lo, hi, dsts):
                nc.scalar.activation(
                    out=ms_all[:, lo:hi, :], in_=ms_all[:, lo:hi, :],
                    func=mybir.ActivationFunctionType.Abs_reciprocal_sqrt,
                    scale=1.0 / DH, bias=eps_sb)
                for slot, dst in dsts:
                    nc.vector.tensor_mul(dst, dst, ms_all[:, slot, :])

            def emit_out(idx, lhs_for_i):
                psum = ps_proj.tile([P, TCH], F32, tag="proj", bufs=2,
                                    name=f"proj{idx}_{j}")
                for i in range(NDCH):
                    nc.tensor.matmul(psum, lhs_for_i(i), xts[i],
                                     start=(i == 0), stop=(i == NDCH - 1))
                    if i == 4 and stats_q:
                        stats_q.pop(0)()
                    if dq and i % 3 == 2:
                        dq.popleft()()
                if idx == 4:  # k
                    rope_raw(idx, psum, krot[:, cs])
                elif idx == 5:  # v: evacuate + transpose blocks + ones col
                    vsb = pre_pool.tile([P, TCH], BF16, tag="vsb", name=f"vsb{j}")
                    nc.vector.tensor_copy(out=vsb, in_=psum)
                    for mm in range(4):
                        m = 4 * j + mm
                        tr = ps_aux.tile([P, P], BF16, tag="aux", name=f"vtr{m}")
                        nc.tensor.transpose(tr, vsb[:, mm * P:(mm + 1) * P], ident)
                        nc.vector.tensor_copy(out=vaug[m][:, 0:DH], in_=tr)
                        nc.vector.memset(vaug[m][:, DH:DH + 1], 1.0)
                else:  # q head
                    rope_raw(idx, psum, qr_tiles[idx])

            def fin():
                while stats_q:
                    stats_q.pop(0)()
                rope_fin(0, 5, [(0, krot[:, cs]), (1, qr_tiles[0]),
                                (2, qr_tiles[1]), (3, qr_tiles[2]),
                                (4, qr_tiles[3])])

            emit_out(4, lambda i: wk_sb[:, i, :])
            if xnext is not None:
                xnext()
            emit_out(5, lambda i: wv_sb[:, i, :])
            return emit_out, fin

        # ---------------- phase D(j): attention + output projection ----------
        # Scores+exp are emitted as per-m closures so the main loop can weave
        # them between projection matmuls: the exp stream then saturates the
        # scalar engine across the whole iteration instead of bursting, and
        # PE (in-order queue) never parks on a psum-bank wait.
        def make_D_scores(j, qr_tiles, h):
            pts = [pt_pool.tile([P, TCH], BF16, tag=f"pt{m}",
                                name=f"pt{m}_{j}_{h}",
                                bufs=(2 if m < 8 else 1))
                   for m in range(4 * j + 4)]

            def mk(m):
                def go():
                    sps = ps_s.tile([P, TCH], F32, tag="s", name=f"s{j}_{h}_{m}")
                    if m >= 4 * j:
                        # diagonal chunk: cols [0:dcol] are never read by any
                        # pv matmul -- skip them in the matmul and the exp.
                        dcol = P * (m - 4 * j)
                        ds_ = slice(dcol, dcol + P)
                        nc.tensor.matmul(sps[:, dcol:TCH],
                                         krot[:, m * P:(m + 1) * P],
                                         qr_tiles[h][:, dcol:TCH],
                                         start=True, stop=True)
                        nc.vector.tensor_add(sps[:, ds_], sps[:, ds_], mask_sb)
                        nc.scalar.activation(
                            out=pts[m][:, dcol:TCH], in_=sps[:, dcol:TCH],
                            func=mybir.ActivationFunctionType.Exp, scale=SCALE)
                    else:
                        nc.tensor.matmul(sps, krot[:, m * P:(m + 1) * P],
                                         qr_tiles[h], start=True, stop=True)
                        nc.scalar.activation(
                            out=pts[m], in_=sps,
                            func=mybir.ActivationFunctionType.Exp, scale=SCALE)
                return go

            return pts, [mk(m) for m in range(4 * j + 4)]

        def emit_D_pv(j, h, pts, yt, dq=None):
            def pv_group(n):
                last = 4 * j + n
                yps = ps_y.tile([P, DH + 1], F32, tag="y", name=f"y{j}_{h}_{n}")
                for m in range(last + 1):
                    nc.tensor.matmul(yps, pts[m][:, n * P:(n + 1) * P], vaug[m],
                                     start=(m == 0), stop=(m == last))
                return yps

            def finish(n, yps):
                # evacuate psum promptly so the y bank frees fast
                ysb = osb_pool.tile([P, DH + 1], F32, tag="ysb",
                                    name=f"ys{j}{h}{n}")
                nc.vector.tensor_copy(out=ysb, in_=yps)
                rinv = osb_pool.tile([P, 1], F32, tag="rinv", name=f"ri{j}{h}{n}")
                nc.vector.reciprocal(out=rinv, in_=ysb[:, DH:DH + 1])
                ynorm = osb_pool.tile([P, P], BF16, tag="ynorm",
                                      name=f"yn{j}{h}{n}")
                nc.vector.tensor_scalar_mul(ynorm, ysb[:, 0:DH], rinv)
                ytr = ps_aux.tile([P, P], BF16, tag="aux", name=f"ytr{j}{h}{n}")
                nc.tensor.transpose(ytr, ynorm, ident)
                nc.vector.tensor_copy(out=yt[h][:, n * P:(n + 1) * P], in_=ytr)

            for n in range(4):
                finish(n, pv_group(n))
                if dq:
                    dq.popleft()()

        def emit_D_out(j, yt, dq=None):
            # out DMAs ride the gpsimd SWDGE ring: the sync HWDGE queue is
            # busy streaming next-chunk inputs, and queueing behind them holds
            # osb (and so the o psum bank) hostage. The last chunk instead
            # uses the by-then-idle sync queue in 512-wide pieces so the tail
            # drain is one small transfer.
            last = j == NTCH - 1
            for n in range(4):
                for half in range(2):
                    osb = osb_pool.tile([P, D // 2], F32, tag="osb", bufs=3,
                                        name=f"osb{j}{n}{half}")
                    for dch in range(2):
                        dc = 2 * half + dch
                        ops = ps_o.tile([P, TCH], F32, tag="o", name=f"o{j}_{n}_{dc}")
                        for h in range(REP):
                            nc.tensor.matmul(
                                ops, yt[h][:, n * P:(n + 1) * P],
                                wo_sb[:, h, dc * TCH:(dc + 1) * TCH],
                                start=(h == 0), stop=(h == REP - 1))
                        nc.vector.tensor_copy(
                            out=osb[:, dch * TCH:(dch + 1) * TCH], in_=ops)
                        if last:
                            nc.sync.dma_start(
                                out=out[j * TCH + n * P: j * TCH + (n + 1) * P,
                                        dc * TCH:(dc + 1) * TCH],
                                in_=osb[:, dch * TCH:(dch + 1) * TCH])
                    if not last:
                        nc.gpsimd.dma_start(
                            out=out[j * TCH + n * P: j * TCH + (n + 1) * P,
                                    half * (D // 2):(half + 1) * (D // 2)],
                            in_=osb)
                    if dq:
                        dq.popleft()()

        # ---------- software pipeline: weave D(j-1) scores into A(j) ---------
        import collections as _c
        qr_all = {}
        yt_all = {}
        for j in range(NTCH):
            qr_all[j] = [
                qr_pool.tile([P, TCH], BF16, tag=f"qr{h}", name=f"qr{h}_{j}")
                for h in range(REP)
            ]
            yt_all[j] = [
                yt_pool.tile([P, TCH], BF16, tag=f"yt{h}", name=f"yt{h}_{j}")
                for h in range(REP)
            ]
            if j == 0:
                xch_all = {0: load_x_chunk(0)}
            dq = _c.deque()
            heads = []  # (h, pts); fed head-serial so pv(h) precedes h+1 m>=8
            cur = [0]
            if j >= 1:
                for h in range(REP):
                    pts, cls = make_D_scores(j - 1, qr_all[j - 1], h)
                    heads.append((h, pts, cls))
                dq.extend(heads[0][2])

            def refill():
                # between passes: if the current head's scores all issued,
                # emit its pv and queue the next head's scores
                while heads and cur[0] < REP and not dq:
                    h, pts, _ = heads[cur[0]]
                    emit_D_pv(j - 1, h, pts, yt_all[j - 1])
                    cur[0] += 1
                    if cur[0] < REP:
                        dq.extend(heads[cur[0]][2])
                        break

            def xnext(j=j):
                if j + 1 < NTCH:
                    xch_all[j + 1] = load_x_chunk(j + 1)
            emit_out, fin = emit_A_kv(j, qr_all[j], dq, xch_all[j],
                                      xnext if j + 1 < NTCH else None)
            refill()
            emit_out(0, lambda i: wq_sb[:, 0 * NDCH + i, :])
            refill()
            emit_out(1, lambda i: wq_sb[:, 1 * NDCH + i, :])
            refill()
            emit_out(2, lambda i: wq_sb[:, 2 * NDCH + i, :])
            refill()
            emit_out(3, lambda i: wq_sb[:, 3 * NDCH + i, :])
            fin()
            # drain any leftover heads: flush scores down to three, then let
            # the pv-group slots pull the diag closures (group n only needs
            # m <= 4(j-1)+n, so three queued closures is always safe)
            while cur[0] < REP and heads:
                h, pts, _ = heads[cur[0]]
                while len(dq) > 3:
                    dq.popleft()()
                emit_D_pv(j - 1, h, pts, yt_all[j - 1], dq=dq)
                while dq:
                    dq.popleft()()
                cur[0] += 1
                if cur[0] < REP:
                    dq.extend(heads[cur[0]][2])
            if j >= 1:
                emit_D_out(j - 1, yt_all[j - 1])

        # tail: D(3) woven into D_out(2)-free slots no longer available --
        # feed its scores through pv slots head-serially
        jL = NTCH - 1
        dq = _c.deque()
        tail_heads = []
        for h in range(REP):
            pts, cls = make_D_scores(jL, qr_all[jL], h)
            tail_heads.append((h, pts, cls))
        for h, pts, cls in tail_heads:
            dq.extend(cls)
            while len(dq) > 3:
                dq.popleft()()
            emit_D_pv(jL, h, pts, yt_all[jL], dq=dq)
            while dq:
                dq.popleft()()
        emit_D_out(jL, yt_all[jL])


def _prep_inputs(x, cos, sin, Wq, Wk, Wv, Wo):
    cosT = np.ascontiguousarray(cos[0, :, 0, :].T.astype(np.float32))  # [64, T]
    sinT = np.ascontiguousarray(sin[0, :, 0, :].T.astype(np.float32))
    cos2 = np.concatenate([cosT, cosT], axis=0).astype(_BF16NP)
    sin2n = np.concatenate([-sinT, sinT], axis=0).astype(_BF16NP)
    in_maps = []
    for i in range(N_CORES):
        b, g = i // 4, i % 4
        in_maps.append({
            "xT": np.ascontiguousarray(x[b].T.astype(_BF16NP)),
            "cos2": cos2,
            "sin2n": sin2n,
            "wq": np.ascontiguousarray(Wq[:, g * HG:(g + 1) * HG].astype(_BF16NP)),
            "wk": np.ascontiguousarray(Wk[:, g * DH:(g + 1) * DH].astype(_BF16NP)),
            "wv": np.ascontiguousarray(Wv[:, g * DH:(g + 1) * DH].astype(_BF16NP)),
            "wo": np.ascontiguousarray(Wo[g * HG:(g + 1) * HG, :].astype(_BF16NP)),
        })
    return in_maps


def bench(x, cos, sin, Wq, Wk, Wv, Wo, iters=20):
    """Device-resident timing of the compiled NEFF via the PJRT path.

    Stages all inputs (and fresh donated output buffers) on the devices
    before each timed call, so the measured wall time is dispatch + execute
    + sync only.
    """
    import time

    import jax
    from jax.sharding import Mesh, PartitionSpec
    from jax.experimental.shard_map import shard_map
    import concourse.bass2jax as bass2jax
    import concourse.mybir as mybir_

    if "nc" not in _CACHE:
        _CACHE["nc"] = _build()
    nc = _CACHE["nc"]
    in_maps = _prep_inputs(
        np.asarray(x), np.asarray(cos), np.asarray(sin),
        np.asarray(Wq), np.asarray(Wk), np.asarray(Wv), np.asarray(Wo))

    bass2jax.install_neuronx_cc_hook()
    partition_name = (
        nc.partition_id_tensor.name if nc.partition_id_tensor else None)
    in_names, out_names, out_avals, zero_outs = [], [], [], []
    for alloc in nc.m.functions[0].allocations:
        if not isinstance(alloc, mybir_.MemoryLocationSet):
            continue
        name = alloc.memorylocations[0].name
        if alloc.kind == "ExternalInput":
            if name != partition_name:
                in_names.append(name)
        elif alloc.kind == "ExternalOutput":
            shape = tuple(alloc.tensor_shape)
            dtype = mybir_.dt.np(alloc.dtype)
            out_names.append(name)
            out_avals.append(jax.core.ShapedArray(shape, dtype))
            zero_outs.append(np.zeros(shape, dtype))
    n_params = len(in_names)
    n_outs = len(out_avals)
    all_names = in_names + out_names
    if partition_name is not None:
        all_names = all_names + [partition_name]

    def _body(*args):
        operands = list(args)
        if partition_name is not None:
            operands.append(bass2jax.partition_id_tensor())
        outs = bass2jax._bass_exec_p.bind(
            *operands,
            out_avals=tuple(out_avals),
            in_names=tuple(all_names),
            out_names=tuple(out_names),
            lowering_input_output_aliases=(),
            sim_require_finite=True,
            sim_require_nnan=True,
            nc=nc,
        )
        return tuple(outs)

    devices = jax.devices()[:N_CORES]
    mesh = Mesh(np.asarray(devices), ("core",))
    donate = tuple(range(n_params, n_params + n_outs))
    sharded = jax.jit(
        shard_map(
            _body, mesh=mesh,
            in_specs=(PartitionSpec("core"),) * (n_params + n_outs),
            out_specs=(PartitionSpec("core"),) * n_outs,
            check_rep=False,
        ),
        donate_argnums=donate, keep_unused=True,
    )
    sharding = jax.sharding.NamedSharding(mesh, PartitionSpec("core"))
    concat_in = [
        jax.device_put(
            np.concatenate([np.asarray(in_maps[c][n]) for c in range(N_CORES)], 0),
            sharding)
        for n in in_names
    ]
    jax.block_until_ready(concat_in)

    def fresh_zeros():
        zs = [
            jax.device_put(
                np.zeros((N_CORES * z.shape[0], *z.shape[1:]), z.dtype), sharding)
            for z in zero_outs
        ]
        jax.block_until_ready(zs)
        return zs

    # warmup (compiles the jit)
    outs = sharded(*concat_in, *fresh_zeros())
    jax.block_until_ready(outs)

    times = []
    for _ in range(iters):
        zs = fresh_zeros()
        t0 = time.perf_counter()
        outs = sharded(*concat_in, *zs)
        jax.block_until_ready(outs)
        times.append(time.perf_counter() - t0)
    times = np.array(times)
    return {
        "min_s": float(times.min()),
        "median_s": float(np.median(times)),
        "mean_s": float(times.mean()),
        "all_s": times.tolist(),
    }


def kernel(x, cos, sin, Wq, Wk, Wv, Wo, _trace_flag=False):
    if "nc" not in _CACHE:
        _CACHE["nc"] = _build()
    nc = _CACHE["nc"]
    in_maps = _prep_inputs(
        np.asarray(x), np.asarray(cos), np.asarray(sin),
        np.asarray(Wq), np.asarray(Wk), np.asarray(Wv), np.asarray(Wo))
    res = run_bass_kernel_spmd(
        nc, in_maps, core_ids=list(range(N_CORES)), trace=_trace_flag)
    _CACHE["last_result"] = res
    out = np.empty((B, T, D), dtype=np.float32)
    for b in range(B):
        acc = res.results[4 * b]["out"].astype(np.float32).copy()
        for g in range(1, 4):
            acc += res.results[4 * b + g]["out"]
        out[b] = acc
    return out

